# revision 30
# baseline (speedup 1.0000x reference)
"""CGConv GNN message-passing layer on 8 Trainium2 NeuronCores.

Strategy (edge-parallel by destination, no collectives):
  * Host sorts edges by destination node and shards them across the 8 cores
    by destination range (6250 nodes / core).  Each core's edges are packed
    into 512-edge "superblocks" whose destinations span <= 64 consecutive
    rows (a window).  The segment-sum is then done ON-CHIP per superblock
    with a one-hot(S)^T @ messages matmul into PSUM, so no cross-core
    reduction and no DRAM read-modify-write scatter is needed.
  * node_hidden is computed replicated on every core (bf16 table in DRAM)
    and gathered per edge with indirect DMA.
  * Weights/most activations in bf16 (PSUM accumulation in fp32); the
    residual node_hidden path is computed with a 3-term bf16 split
    (fp32r-style) for fp32-level accuracy.

Self-contained: hardcodes all shapes; imports concourse from /opt/trn_rl_repo.
"""

import sys

if "/opt/trn_rl_repo" not in sys.path:
    sys.path.insert(0, "/opt/trn_rl_repo")

import numpy as np
import ml_dtypes

BF16 = ml_dtypes.bfloat16

# problem constants
N_NODES = 50000
N_EDGES = 800000
D = 128
BN_EPS = 1e-3

# sharding / layout constants
N_CORES = 8
NPC = N_NODES // N_CORES          # 6250 nodes per core
SB = 512                          # edge slots per superblock
W = 64                            # max window rows per superblock
SB_COUNT = 208                    # superblocks per core (13 groups of 16)
SCAT_G = 16                       # superblocks per output scatter group
N_GROUPS = SB_COUNT // SCAT_G     # 13
E_CAP = SB_COUNT * SB             # 106496 edge slots per core
N_PAD = 50176                     # node_features padded rows (392*128)
# nh PAIR table: row k = [nh[2k], nh[2k+1]] (512B rows), row 0 = zeros so
# int16 dma_gather indices (src>>1)+1 address the whole 50k-node table
N_TAB = N_PAD // 2 + 2            # 25090 pair rows (+ zero slot + spare)
OWN_PAD = 6656                    # 13*512 padded own-node rows

_CACHE = {}


def _build_program():
    import concourse.bass as bass
    import concourse.mybir as mybir
    import concourse.tile as tile
    from concourse import bacc

    f32 = mybir.dt.float32
    bf16 = mybir.dt.bfloat16
    i32 = mybir.dt.int32
    i16 = mybir.dt.int16
    AF = mybir.ActivationFunctionType
    ALU = mybir.AluOpType

    nc = bacc.Bacc("TRN2", target_bir_lowering=False, debug=True)

    # ---------------- DRAM I/O ----------------
    def inp(name, shape, dt):
        return nc.dram_tensor(name, shape, dt, kind="ExternalInput")

    # shared across cores
    nf = inp("nf", [N_PAD, D], bf16)
    wg = inp("wg", [D, D], bf16)
    wf = inp("wf", [D, D], bf16)
    wn_hi = inp("wn_hi", [D, D], bf16)
    wn_lo = inp("wn_lo", [D, D], bf16)
    bg_col_d = inp("bg_col", [D, 1], f32)
    bnode_col_d = inp("bnode_col", [D, 1], f32)
    bns_col_d = inp("bns_col", [D, 1], f32)
    bnb_col_d = inp("bnb_col", [D, 1], f32)
    bg4_row_d = inp("bg4_row", [1, SB], bf16)
    bf4_row_d = inp("bf4_row", [1, SB], bf16)
    bn4_row_d = inp("bn4_row", [1, SB], bf16)
    ones_row_d = inp("ones_row", [1, SB], bf16)
    id128b_d = inp("id128b", [D, D], bf16)
    id128f_d = inp("id128f", [D, D], f32)
    # per-core
    ef = inp("ef", [E_CAP, D], bf16)
    nf_own_hi = inp("nf_own_hi", [OWN_PAD, D], bf16)
    nf_own_lo = inp("nf_own_lo", [OWN_PAD, D], bf16)
    gidx_a = inp("gidx_a", [D, SB_COUNT * 32], i16)
    s_a = inp("s_a", [SB_COUNT, D, 4, 2, W], bf16)
    sidx_a = inp("sidx_a", [D, OWN_PAD // D], i32)
    wscale_a = inp("wscale_a", [W, SB_COUNT], f32)

    out_d = nc.dram_tensor("out", [OWN_PAD, D], f32, kind="ExternalOutput")
    nhtab = nc.dram_tensor("nhtab", [N_TAB, 2 * D], bf16)
    stage = nc.dram_tensor("stage", [SB_COUNT * W, D], f32)

    out_r = out_d.rearrange("(b c p) j -> b p c j", c=4, p=128)

    def nhtab_block(nb):
        # block nb covers nodes [nb*512, nb*512+512) = pair rows nb*256+1..+256;
        # sbuf partition p=2*p2+h maps to (row p2, half h): merged stride is a
        # plain 128-element step, so this is a regular strided AP
        v = nhtab[nb * 256 + 1:nb * 256 + 257]
        return v.rearrange("(c p2) (h j) -> (p2 h) c j", c=4, p2=64, h=2, j=D)

    NB1 = N_PAD // SB          # 98 node blocks (table)
    NB3 = OWN_PAD // SB        # 13 own blocks

    with tile.TileContext(nc) as tc:
        with (
            tc.tile_pool(name="const", bufs=1) as constp,
            tc.tile_pool(name="big", bufs=1) as bigp,
            tc.tile_pool(name="ldA", bufs=4) as ldA,
            tc.tile_pool(name="ldB", bufs=4) as ldB,
            tc.tile_pool(name="mid", bufs=4) as midp,
            tc.tile_pool(name="msg", bufs=4) as msgp,
            tc.tile_pool(name="aggp", bufs=4) as aggp,
            tc.tile_pool(name="psA", bufs=3, space="PSUM") as psA,
            tc.tile_pool(name="psB", bufs=3, space="PSUM") as psB,
            tc.tile_pool(name="psD", bufs=2, space="PSUM") as psD,
        ):
            # ---------- persistent constants in SBUF ----------
            def load_const(dram, shape, dt, tag):
                t = constp.tile(shape, dt, tag=tag)
                nc.sync.dma_start(t[:], dram[:])
                return t

            wg_s = load_const(wg, [D, D], bf16, "wg")
            wf_s = load_const(wf, [D, D], bf16, "wf")
            wnh_s = load_const(wn_hi, [D, D], bf16, "wnh")
            wnl_s = load_const(wn_lo, [D, D], bf16, "wnl")
            bg_s = load_const(bg_col_d, [D, 1], f32, "bg")
            bnode_s = load_const(bnode_col_d, [D, 1], f32, "bnode")
            bns_s = load_const(bns_col_d, [D, 1], f32, "bns")
            bnb_s = load_const(bnb_col_d, [D, 1], f32, "bnb")
            bg4_s = load_const(bg4_row_d, [1, SB], bf16, "bg4")
            bf4_s = load_const(bf4_row_d, [1, SB], bf16, "bf4")
            bn4_s = load_const(bn4_row_d, [1, SB], bf16, "bn4")
            ones_s = load_const(ones_row_d, [1, SB], bf16, "ones")
            id128b_s = load_const(id128b_d, [D, D], bf16, "id128b")
            id128f_s = load_const(id128f_d, [D, D], f32, "id128f")
            gidx_s = load_const(gidx_a, [D, SB_COUNT * 32], i16, "gidx")
            sidx_s = load_const(sidx_a, [D, OWN_PAD // D], i32, "sidx")
            wscale_s = load_const(wscale_a, [W, SB_COUNT], f32, "wscale")

            nhT_own = bigp.tile([D, OWN_PAD], f32, tag="nhT_own")

            # ---------- zero-init regions ----------
            zb = constp.tile([2, 2 * D], bf16, tag="zb")
            nc.vector.memset(zb[:], 0.0)
            nc.sync.dma_start(nhtab[0:1, :], zb[:1, :])              # zero slot
            nc.sync.dma_start(nhtab[N_TAB - 1:N_TAB, :], zb[:1, :])  # spare row

            # ---------- phase 1a: node_hidden table (bf16, replicated) ----------
            for nb in range(NB1):
                nfT = ldA.tile([D, SB], bf16, tag="nfT")
                nc.sync.dma_start_transpose(nfT[:], nf[nb * SB:(nb + 1) * SB, :])
                ps = psA.tile([128, 4, 128], f32, tag="psA")
                psv = ps[:].rearrange("p c j -> p (c j)")
                # bias first (start=True clears), then 4 accumulate chunks
                nc.tensor.matmul(psv, lhsT=ones_s[:, :D], rhs=bn4_s[:],
                                 start=True, stop=False, skip_group_check=True)
                for c in range(4):
                    nc.tensor.matmul(ps[:, c, :], lhsT=nfT[:, c * 128:(c + 1) * 128],
                                     rhs=wnh_s[:], start=False, stop=(c == 3),
                                     skip_group_check=True)
                nh_sb = ldB.tile([128, 4, 128], bf16, tag="nh_sb")
                nc.scalar.activation(nh_sb[:].rearrange("p c j -> p (c j)"), psv, AF.Copy)
                nc.sync.dma_start(nhtab_block(nb), nh_sb[:])

            # ---------- phase 1b: own-slice node_hidden, fp32r, transposed ----------
            for fb in range(NB3):
                nfTh = ldA.tile([D, SB], bf16, tag="nfTh")
                nc.sync.dma_start_transpose(nfTh[:], nf_own_hi[fb * SB:(fb + 1) * SB, :])
                nfTl = ldA.tile([D, SB], bf16, tag="nfTl")
                nc.sync.dma_start_transpose(nfTl[:], nf_own_lo[fb * SB:(fb + 1) * SB, :])
                ps = psB.tile([D, SB], f32, tag="psB")
                nc.tensor.matmul(ps[:], lhsT=wnh_s[:], rhs=nfTh[:], start=True, stop=False)
                nc.tensor.matmul(ps[:], lhsT=wnl_s[:], rhs=nfTh[:], start=False, stop=False)
                nc.tensor.matmul(ps[:], lhsT=wnh_s[:], rhs=nfTl[:], start=False, stop=True)
                nc.scalar.activation(nhT_own[:, fb * SB:(fb + 1) * SB], ps[:],
                                     AF.Identity, bias=bnode_s[:, :1])

            # ---------- phase 2: edges ----------
            for sb in range(SB_COUNT):
                if True:
                    efT = ldA.tile([D, SB], bf16, tag="efT")
                    nc.sync.dma_start_transpose(efT[:], ef[sb * SB:(sb + 1) * SB, :])
                    ps_g = psA.tile([128, 4, 128], f32, tag="psA")
                    ps_f = psB.tile([128, 4, 128], f32, tag="psB")
                    psg_v = ps_g[:].rearrange("p c j -> p (c j)")
                    psf_v = ps_f[:].rearrange("p c j -> p (c j)")
                    nc.tensor.matmul(psg_v, lhsT=ones_s[:, :D], rhs=bg4_s[:],
                                     start=True, stop=False, skip_group_check=True)
                    nc.tensor.matmul(psf_v, lhsT=ones_s[:, :D], rhs=bf4_s[:],
                                     start=True, stop=False, skip_group_check=True)
                    for c in range(4):
                        eslc = efT[:, c * 128:(c + 1) * 128]
                        nc.tensor.matmul(ps_g[:, c, :], lhsT=eslc, rhs=wg_s[:],
                                         start=False, stop=(c == 3), skip_group_check=True)
                        nc.tensor.matmul(ps_f[:, c, :], lhsT=eslc, rhs=wf_s[:],
                                         start=False, stop=(c == 3), skip_group_check=True)
                    gate = midp.tile([D, SB], f32, tag="gate")
                    nc.scalar.activation(gate[:], psg_v, AF.Sigmoid)
                    gf = midp.tile([128, 4, 128], bf16, tag="gf")
                    nc.vector.tensor_tensor(gf[:].rearrange("p c j -> p (c j)"),
                                            gate[:], psf_v, ALU.mult)
                    nbr2 = msgp.tile([128, 4, 2 * D], bf16, tag="nbr2")
                    nc.gpsimd.dma_gather(nbr2[:], nhtab[:],
                                         gidx_s[:, sb * 32:(sb + 1) * 32],
                                         SB, SB, 2 * D, single_packet=False)
                    msg = msgp.tile([128, 4, 2, D], bf16, tag="msg")
                    for h in range(2):
                        nc.vector.tensor_tensor(
                            msg[:, :, h, :], gf[:],
                            nbr2[:].rearrange("p c (h j) -> p c h j", h=2)[:, :, h, :],
                            ALU.mult)
                    S = msgp.tile([128, 4, 2, W], bf16, tag="S")
                    nc.scalar.dma_start(S[:], s_a[sb])
                    ps_w = psD.tile([W, D], f32, tag="psD")
                    for c in range(4):
                        for h in range(2):
                            nc.tensor.matmul(ps_w[:], lhsT=S[:, c, h, :],
                                             rhs=msg[:, c, h, :],
                                             start=(c == 0 and h == 0),
                                             stop=(c == 3 and h == 1))
                    agg = aggp.tile([W, D], f32, tag="agg")
                    nc.vector.tensor_scalar_mul(agg[:], ps_w[:],
                                                wscale_s[:, sb:sb + 1])
                    nc.scalar.dma_start(stage[sb * W:(sb + 1) * W, :], agg[:])

            # ---------- phase 3: mean+residual+BN+relu ----------
            for fb in range(NB3):
                accf = ldB.tile([128, 4, 128], f32, tag="accf")
                for c in range(4):
                    nc.gpsimd.indirect_dma_start(
                        out=accf[:, c, :], out_offset=None, in_=stage[:],
                        in_offset=bass.IndirectOffsetOnAxis(
                            ap=sidx_s[:, fb * 4 + c:fb * 4 + c + 1], axis=0))
                ps_aT = psA.tile([D, SB], f32, tag="psA")
                for c in range(4):
                    nc.tensor.transpose(ps_aT[:, c * 128:(c + 1) * 128], accf[:, c, :],
                                        id128f_s[:])
                t_sb = midp.tile([D, SB], f32, tag="t_sb")
                nc.vector.tensor_tensor(t_sb[:], ps_aT[:],
                                        nhT_own[:, fb * SB:(fb + 1) * SB], ALU.add)
                oT = midp.tile([D, SB], f32, tag="oT")
                nc.scalar.activation(oT[:], t_sb[:], AF.Relu,
                                     bias=bnb_s[:, :1], scale=bns_s[:, :1])
                ps_o = psB.tile([128, 4, 128], f32, tag="psB")
                for c in range(4):
                    nc.tensor.transpose(ps_o[:, c, :], oT[:, c * 128:(c + 1) * 128],
                                        id128f_s[:])
                o_sb = ldB.tile([128, 4, 128], f32, tag="o_sb")
                nc.vector.tensor_copy(o_sb[:], ps_o[:])
                nc.sync.dma_start(out_r[fb], o_sb[:])

    nc.compile()
    return nc


def _host_prep(inputs):
    """Sort/shard/pack edges; build all per-core and shared input arrays."""
    nf32 = np.asarray(inputs["node_features"], np.float32)
    ef32 = np.asarray(inputs["edge_features"], np.float32)
    eidx = np.asarray(inputs["edge_indices"], np.int32)
    dst = eidx[:, 0].astype(np.int64)
    src = eidx[:, 1].astype(np.int64)

    order = np.argsort(dst, kind="stable")
    dst_s = dst[order]
    src_s = src[order]
    deg = np.bincount(dst, minlength=N_NODES).astype(np.int64)
    row_start = np.zeros(N_NODES + 1, np.int64)
    np.cumsum(deg, out=row_start[1:])
    scale = np.where(deg > 0, 1.0 / np.maximum(deg, 1), 0.0).astype(np.float32)

    ef_bf = ef32.astype(BF16)

    # shared arrays
    nf_pad = np.zeros((N_PAD, D), BF16)
    nf_pad[:N_NODES] = nf32.astype(BF16)
    wn_hi = np.asarray(inputs["W_node"], np.float32).astype(BF16)
    wn_lo = (np.asarray(inputs["W_node"], np.float32)
             - wn_hi.astype(np.float32)).astype(BF16)
    gamma = np.asarray(inputs["bn_gamma"], np.float32)
    var = np.asarray(inputs["bn_var"], np.float32)
    mean = np.asarray(inputs["bn_mean"], np.float32)
    beta = np.asarray(inputs["bn_beta"], np.float32)
    bns = gamma / np.sqrt(var + BN_EPS)
    bnb = beta - mean * bns
    b_node = np.asarray(inputs["b_node"], np.float32)

    shared = {
        "nf": nf_pad,
        "wg": np.asarray(inputs["W_gate"], np.float32).astype(BF16),
        "wf": np.asarray(inputs["W_filt"], np.float32).astype(BF16),
        "wn_hi": wn_hi,
        "wn_lo": wn_lo,
        "bg_col": np.asarray(inputs["b_gate"], np.float32).reshape(D, 1),
        "bnode_col": b_node.reshape(D, 1),
        "bns_col": bns.reshape(D, 1).astype(np.float32),
        "bnb_col": bnb.reshape(D, 1).astype(np.float32),
        "bg4_row": np.tile(np.asarray(inputs["b_gate"], np.float32), 4).reshape(1, SB).astype(BF16),
        "bf4_row": np.tile(np.asarray(inputs["b_filt"], np.float32), 4).reshape(1, SB).astype(BF16),
        "bn4_row": np.tile(b_node, 4).reshape(1, SB).astype(BF16),
        "ones_row": np.ones((1, SB), BF16),
        "id128b": np.eye(D, dtype=np.float32).astype(BF16),
        "id128f": np.eye(D, dtype=np.float32),
    }

    in_maps = []
    for c in range(N_CORES):
        r0, r1 = c * NPC, (c + 1) * NPC
        # ---- greedy window packing: whole rows, <=SB slots, <=W rows ----
        sbs_rows = []
        cur_rows = []
        cur_slots = 0
        for r in range(r0, r1):
            dg = int(deg[r])
            assert dg <= SB
            if cur_rows and (cur_slots + dg > SB or len(cur_rows) >= W):
                sbs_rows.append(cur_rows)
                cur_rows, cur_slots = [], 0
            cur_rows.append(r)
            cur_slots += dg
        if cur_rows:
            sbs_rows.append(cur_rows)
        assert len(sbs_rows) <= SB_COUNT, f"core {c}: {len(sbs_rows)} superblocks"

        ef_idx = np.full((SB_COUNT, SB), -1, np.int64)
        rel = np.zeros((SB_COUNT, SB), np.int16)
        srcs = np.full((SB_COUNT, SB), -1, np.int64)  # -1 = pad slot
        wrow = np.full((SB_COUNT, W), OWN_PAD - 1, np.int64)  # pad -> garbage row
        wscale = np.zeros((SB_COUNT, W), np.float32)
        for si, rows in enumerate(sbs_rows):
            pos = 0
            for j, r in enumerate(rows):
                wrow[si, j] = r - r0
                wscale[si, j] = scale[r]
                e0, e1 = row_start[r], row_start[r + 1]
                n = int(e1 - e0)
                if n:
                    ef_idx[si, pos:pos + n] = order[e0:e1]
                    rel[si, pos:pos + n] = j
                    srcs[si, pos:pos + n] = src_s[e0:e1]
                    pos += n

        ef_core = np.zeros((E_CAP, D), BF16)
        flat_idx = ef_idx.reshape(-1)
        valid = flat_idx >= 0
        ef_core[valid] = ef_bf[flat_idx[valid]]

        # split-table gather indices (+1 zero-slot encoding), wrapped
        # [16, 32] per sb then replicated to 128 partitions
        def wrap16(v):  # [SB_COUNT, SB] -> [128, SB_COUNT*32]
            w16 = v.reshape(SB_COUNT, 32, 16).transpose(0, 2, 1)
            return np.tile(w16, (1, 8, 1)).transpose(1, 0, 2).reshape(
                128, SB_COUNT * 32).copy()

        valid = srcs >= 0
        gidx = np.where(valid, (srcs >> 1) + 1, 0).astype(np.int16)
        # parity-split one-hot S matrices [sb, p, c, h, w]: edge selects its
        # pair-half h = src&1; invalid/pad slots get all-zero rows
        rel_pc = rel.reshape(SB_COUNT, 4, 128).transpose(0, 2, 1)      # [sb,p,c]
        par_pc = (srcs & 1).reshape(SB_COUNT, 4, 128).transpose(0, 2, 1)
        val_pc = valid.reshape(SB_COUNT, 4, 128).transpose(0, 2, 1)
        s_host = np.zeros((SB_COUNT, 128, 4, 2, W), np.float32)
        sbi, pi, ci = np.nonzero(val_pc)
        s_host[sbi, pi, ci, par_pc[sbi, pi, ci], rel_pc[sbi, pi, ci]] = 1.0
        s_host = s_host.astype(BF16)

        nf_own_hi = np.zeros((OWN_PAD, D), BF16)
        nf_own_hi[:NPC] = nf32[r0:r1].astype(BF16)
        nf_own_lo = np.zeros((OWN_PAD, D), BF16)
        nf_own_lo[:NPC] = (nf32[r0:r1]
                           - nf_own_hi[:NPC].astype(np.float32)).astype(BF16)

        # phase-3 staging gather idx: node (local) r -> its staging row sb*W+j
        stage_idx = np.zeros(OWN_PAD, np.int32)
        for si, rows in enumerate(sbs_rows):
            for j, r in enumerate(rows):
                stage_idx[r - r0] = si * W + j
        sidx = stage_idx.reshape(OWN_PAD // D, D).T.copy()  # [p, chunk]

        m = dict(shared)
        m.update({
            "ef": ef_core,
            "nf_own_hi": nf_own_hi,
            "nf_own_lo": nf_own_lo,
            "gidx_a": wrap16(gidx).astype(np.int16),
            "s_a": s_host,
            "sidx_a": sidx.astype(np.int32),
            "wscale_a": wscale.T.copy(),
        })
        in_maps.append(m)
    return in_maps


def get_program():
    if "nc" not in _CACHE:
        _CACHE["nc"] = _build_program()
    return _CACHE["nc"]


def kernel(**inputs) -> np.ndarray:
    from concourse.bass_utils import run_bass_kernel_spmd

    nc = get_program()
    in_maps = _host_prep(inputs)
    res = run_bass_kernel_spmd(nc, in_maps, core_ids=list(range(N_CORES)))
    _CACHE["last_exec_time_ns"] = res.exec_time_ns
    out = np.concatenate(
        [np.asarray(res.results[c]["out"])[:NPC] for c in range(N_CORES)], axis=0)
    return out.astype(np.float32)


# revision 34
# speedup vs baseline: 1.2943x; 1.2943x over previous
"""CGConv GNN message-passing layer on 8 Trainium2 NeuronCores.

Strategy (edge-parallel by destination, no collectives):
  * Host sorts edges by destination node and shards them across the 8 cores
    by destination range (6250 nodes / core).  Each core's edges are packed
    into 512-edge "superblocks" whose destinations span <= 64 consecutive
    rows (a window).  The segment-sum is then done ON-CHIP per superblock
    with a one-hot(S)^T @ messages matmul into PSUM, so no cross-core
    reduction and no DRAM read-modify-write scatter is needed.
  * node_hidden is computed replicated on every core (bf16 table in DRAM)
    and gathered per edge with indirect DMA.
  * Weights/most activations in bf16 (PSUM accumulation in fp32); the
    residual node_hidden path is computed with a 3-term bf16 split
    (fp32r-style) for fp32-level accuracy.

Self-contained: hardcodes all shapes; imports concourse from /opt/trn_rl_repo.
"""

import sys

if "/opt/trn_rl_repo" not in sys.path:
    sys.path.insert(0, "/opt/trn_rl_repo")

import numpy as np
import ml_dtypes

BF16 = ml_dtypes.bfloat16

# problem constants
N_NODES = 50000
N_EDGES = 800000
D = 128
BN_EPS = 1e-3

# sharding / layout constants
N_CORES = 8
NPC = N_NODES // N_CORES          # 6250 nodes per core
SB = 512                          # edge slots per superblock
W = 64                            # max window rows per superblock
SB_COUNT = 208                    # superblocks per core (13 groups of 16)
SCAT_G = 16                       # superblocks per output scatter group
N_GROUPS = SB_COUNT // SCAT_G     # 13
E_CAP = SB_COUNT * SB             # 106496 edge slots per core
N_PAD = 50176                     # node_features padded rows (392*128)
# nh PAIR table: row k = [nh[2k], nh[2k+1]] (512B rows), row 0 = zeros so
# int16 dma_gather indices (src>>1)+1 address the whole 50k-node table
N_TAB = N_PAD // 2 + 2            # 25090 pair rows (+ zero slot + spare)
OWN_PAD = 6656                    # 13*512 padded own-node rows

_CACHE = {}


def _build_program():
    import concourse.bass as bass
    import concourse.mybir as mybir
    import concourse.tile as tile
    from concourse import bacc

    f32 = mybir.dt.float32
    bf16 = mybir.dt.bfloat16
    i32 = mybir.dt.int32
    i16 = mybir.dt.int16
    AF = mybir.ActivationFunctionType
    ALU = mybir.AluOpType

    nc = bacc.Bacc("TRN2", target_bir_lowering=False, debug=True)

    # ---------------- DRAM I/O ----------------
    def inp(name, shape, dt):
        return nc.dram_tensor(name, shape, dt, kind="ExternalInput")

    # shared across cores
    nf = inp("nf", [N_PAD, D], bf16)
    wg = inp("wg", [D, D], bf16)
    wf = inp("wf", [D, D], bf16)
    wn_hi = inp("wn_hi", [D, D], bf16)
    wn_lo = inp("wn_lo", [D, D], bf16)
    bg_col_d = inp("bg_col", [D, 1], f32)
    bnode_col_d = inp("bnode_col", [D, 1], f32)
    bns_col_d = inp("bns_col", [D, 1], f32)
    bnb_col_d = inp("bnb_col", [D, 1], f32)
    bf_col_d = inp("bf_col", [D, 1], f32)
    bn4_row_d = inp("bn4_row", [1, SB], bf16)
    ones_row_d = inp("ones_row", [1, SB], bf16)
    id128b_d = inp("id128b", [D, D], bf16)
    id128f_d = inp("id128f", [D, D], f32)
    # per-core
    ef = inp("ef", [E_CAP, D], bf16)
    nf_own_hi = inp("nf_own_hi", [OWN_PAD, D], bf16)
    nf_own_lo = inp("nf_own_lo", [OWN_PAD, D], bf16)
    gidx_a = inp("gidx_a", [D, SB_COUNT * 32], i16)
    s_a = inp("s_a", [SB_COUNT, D, 4, 2, W], bf16)
    sidx_a = inp("sidx_a", [D, OWN_PAD // D], i32)
    wscale_a = inp("wscale_a", [W, SB_COUNT], f32)

    out_d = nc.dram_tensor("out", [OWN_PAD, D], f32, kind="ExternalOutput")
    nhtab = nc.dram_tensor("nhtab", [N_TAB, 2 * D], bf16)
    stage = nc.dram_tensor("stage", [SB_COUNT * W, D], f32)

    out_r = out_d.rearrange("(b c p) j -> b p c j", c=4, p=128)

    def nhtab_block(nb):
        # block nb covers nodes [nb*512, nb*512+512) = pair rows nb*256+1..+256;
        # sbuf partition p=2*p2+h maps to (row p2, half h): merged stride is a
        # plain 128-element step, so this is a regular strided AP
        v = nhtab[nb * 256 + 1:nb * 256 + 257]
        return v.rearrange("(c p2) (h j) -> (p2 h) c j", c=4, p2=64, h=2, j=D)

    NB1 = N_PAD // SB          # 98 node blocks (table)
    NB3 = OWN_PAD // SB        # 13 own blocks

    with tile.TileContext(nc) as tc:
        with (
            tc.tile_pool(name="const", bufs=1) as constp,
            tc.tile_pool(name="big", bufs=1) as bigp,
            tc.tile_pool(name="ldA", bufs=4) as ldA,
            tc.tile_pool(name="ldB", bufs=4) as ldB,
            tc.tile_pool(name="mid", bufs=4) as midp,
            tc.tile_pool(name="msg", bufs=4) as msgp,
            tc.tile_pool(name="aggp", bufs=4) as aggp,
            tc.tile_pool(name="psA", bufs=2, space="PSUM") as psA,
            tc.tile_pool(name="psB", bufs=2, space="PSUM") as psB,
            tc.tile_pool(name="psC", bufs=2, space="PSUM") as psC,
            tc.tile_pool(name="psD", bufs=2, space="PSUM") as psD,
        ):
            # ---------- persistent constants in SBUF ----------
            def load_const(dram, shape, dt, tag):
                t = constp.tile(shape, dt, tag=tag)
                nc.sync.dma_start(t[:], dram[:])
                return t

            wg_s = load_const(wg, [D, D], bf16, "wg")
            wf_s = load_const(wf, [D, D], bf16, "wf")
            wnh_s = load_const(wn_hi, [D, D], bf16, "wnh")
            wnl_s = load_const(wn_lo, [D, D], bf16, "wnl")
            bg_s = load_const(bg_col_d, [D, 1], f32, "bg")
            bnode_s = load_const(bnode_col_d, [D, 1], f32, "bnode")
            bns_s = load_const(bns_col_d, [D, 1], f32, "bns")
            bnb_s = load_const(bnb_col_d, [D, 1], f32, "bnb")
            bf_col_s = load_const(bf_col_d, [D, 1], f32, "bfc")
            bn4_s = load_const(bn4_row_d, [1, SB], bf16, "bn4")
            ones_s = load_const(ones_row_d, [1, SB], bf16, "ones")
            id128b_s = load_const(id128b_d, [D, D], bf16, "id128b")
            id128f_s = load_const(id128f_d, [D, D], f32, "id128f")
            gidx_s = load_const(gidx_a, [D, SB_COUNT * 32], i16, "gidx")
            sidx_s = load_const(sidx_a, [D, OWN_PAD // D], i32, "sidx")
            wscale_s = load_const(wscale_a, [W, SB_COUNT], f32, "wscale")

            nhT_own = bigp.tile([D, OWN_PAD], f32, tag="nhT_own")

            # ---------- zero-init regions ----------
            zb = constp.tile([2, 2 * D], bf16, tag="zb")
            nc.vector.memset(zb[:], 0.0)
            nc.sync.dma_start(nhtab[0:1, :], zb[:1, :])              # zero slot
            nc.sync.dma_start(nhtab[N_TAB - 1:N_TAB, :], zb[:1, :])  # spare row

            # ---------- phase 1a: node_hidden table (bf16, replicated) ----------
            for nb in range(NB1):
                ld_eng = nc.scalar if nb % 2 else nc.sync
                st_eng = nc.sync if nb % 2 else nc.scalar
                psp = psA if nb % 2 else psB
                nfT = ldA.tile([D, SB], bf16, tag="nfT")
                ld_eng.dma_start_transpose(nfT[:], nf[nb * SB:(nb + 1) * SB, :])
                ps = psp.tile([128, 4, 128], f32, tag="psA" if nb % 2 else "psB")
                psv = ps[:].rearrange("p c j -> p (c j)")
                # bias first (start=True clears), then 4 accumulate chunks
                nc.tensor.matmul(psv, lhsT=ones_s[:, :D], rhs=bn4_s[:],
                                 start=True, stop=False, skip_group_check=True)
                for c in range(4):
                    nc.tensor.matmul(ps[:, c, :], lhsT=nfT[:, c * 128:(c + 1) * 128],
                                     rhs=wnh_s[:], start=False, stop=(c == 3),
                                     skip_group_check=True)
                nh_sb = ldB.tile([128, 4, 128], bf16, tag="nh_sb")
                if nb % 2:
                    nc.scalar.activation(nh_sb[:].rearrange("p c j -> p (c j)"), psv,
                                         AF.Copy)
                else:
                    nc.vector.tensor_copy(nh_sb[:].rearrange("p c j -> p (c j)"), psv)
                st_eng.dma_start(nhtab_block(nb), nh_sb[:])

            # ---------- phase 1b emitted inside phase 2 (fills idle slots) ----------
            def emit_ph1b(fb):
                nfTh = ldA.tile([D, SB], bf16, tag="nfTh")
                nc.scalar.dma_start_transpose(nfTh[:], nf_own_hi[fb * SB:(fb + 1) * SB, :])
                nfTl = ldA.tile([D, SB], bf16, tag="nfTl")
                nc.scalar.dma_start_transpose(nfTl[:], nf_own_lo[fb * SB:(fb + 1) * SB, :])
                ps = psB.tile([D, SB], f32, tag="psB")
                nc.tensor.matmul(ps[:], lhsT=wnh_s[:], rhs=nfTh[:], start=True, stop=False)
                nc.tensor.matmul(ps[:], lhsT=wnl_s[:], rhs=nfTh[:], start=False, stop=False)
                nc.tensor.matmul(ps[:], lhsT=wnh_s[:], rhs=nfTl[:], start=False, stop=True)
                nc.scalar.activation(nhT_own[:, fb * SB:(fb + 1) * SB], ps[:],
                                     AF.Identity, bias=bnode_s[:, :1])

            # ---------- phase 2: edges ----------
            for sb in range(SB_COUNT):
                    if sb % 4 == 0:
                        q = sb // 4
                        nbr2 = msgp.tile([128, 16, 2 * D], bf16, tag="nbr2")
                        nc.gpsimd.dma_gather(nbr2[:], nhtab[:],
                                             gidx_s[:, q * 128:(q + 1) * 128],
                                             4 * SB, 4 * SB, 2 * D,
                                             single_packet=False)
                    if sb % 16 == 5 and sb // 16 < NB3:
                        emit_ph1b(sb // 16)
                    efT = ldA.tile([D, SB], bf16, tag="efT")
                    nc.sync.dma_start_transpose(efT[:], ef[sb * SB:(sb + 1) * SB, :])
                    ps_g = psA.tile([D, SB], f32, tag="psA")
                    nc.tensor.matmul(ps_g[:].rearrange("p (c j) -> p c j", c=4),
                                     lhsT=wg_s[:], rhs=efT[:].rearrange("p (c j) -> p c j", c=4),
                                     start=True, stop=True)
                    ps_f = psB.tile([D, SB], f32, tag="psB")
                    nc.tensor.matmul(ps_f[:], lhsT=wf_s[:], rhs=efT[:], start=True, stop=True)
                    gate = midp.tile([D, SB], bf16, tag="gate")
                    nc.scalar.activation(gate[:], ps_g[:], AF.Sigmoid, bias=bg_s[:, :1])
                    filt = midp.tile([D, SB], bf16, tag="filt")
                    nc.vector.tensor_scalar_add(filt[:], ps_f[:], bf_col_s[:, :1])
                    gf = midp.tile([D, SB], bf16, tag="gf")
                    nc.vector.tensor_tensor(gf[:], gate[:], filt[:], ALU.mult)
                    ps_t = psC.tile([128, 4, 128], bf16, tag="psC")
                    for c in range(4):
                        nc.tensor.transpose(ps_t[:, c, :], gf[:, c * 128:(c + 1) * 128],
                                            id128b_s[:])
                    sl = sb % 4
                    nbr_v = nbr2[:, sl * 4:(sl + 1) * 4, :]
                    msg = msgp.tile([128, 4, 2, D], bf16, tag="msg")
                    for h in range(2):
                        nc.vector.tensor_tensor(
                            msg[:, :, h, :], ps_t[:],
                            nbr_v.rearrange("p c (h j) -> p c h j", h=2)[:, :, h, :],
                            ALU.mult)
                    S = msgp.tile([128, 4, 2, W], bf16, tag="S")
                    nc.scalar.dma_start(S[:], s_a[sb])
                    ps_w = psD.tile([W, D], f32, tag="psD")
                    for c in range(4):
                        for h in range(2):
                            nc.tensor.matmul(ps_w[:], lhsT=S[:, c, h, :],
                                             rhs=msg[:, c, h, :],
                                             start=(c == 0 and h == 0),
                                             stop=(c == 3 and h == 1))
                    agg = aggp.tile([W, D], f32, tag="agg")
                    nc.vector.tensor_scalar_mul(agg[:], ps_w[:],
                                                wscale_s[:, sb:sb + 1])
                    nc.sync.dma_start(stage[sb * W:(sb + 1) * W, :], agg[:])

            # ---------- phase 3: mean+residual+BN+relu ----------
            for fb in range(NB3):
                accf = ldB.tile([128, 4, 128], f32, tag="accf")
                for c in range(4):
                    nc.gpsimd.indirect_dma_start(
                        out=accf[:, c, :], out_offset=None, in_=stage[:],
                        in_offset=bass.IndirectOffsetOnAxis(
                            ap=sidx_s[:, fb * 4 + c:fb * 4 + c + 1], axis=0))
                ps_aT = psA.tile([D, SB], f32, tag="psA")
                for c in range(4):
                    nc.tensor.transpose(ps_aT[:, c * 128:(c + 1) * 128], accf[:, c, :],
                                        id128f_s[:])
                t_sb = midp.tile([D, SB], f32, tag="t_sb")
                nc.vector.tensor_tensor(t_sb[:], ps_aT[:],
                                        nhT_own[:, fb * SB:(fb + 1) * SB], ALU.add)
                oT = midp.tile([D, SB], f32, tag="oT")
                nc.scalar.activation(oT[:], t_sb[:], AF.Relu,
                                     bias=bnb_s[:, :1], scale=bns_s[:, :1])
                ps_o = psB.tile([128, 4, 128], f32, tag="psB")
                for c in range(4):
                    nc.tensor.transpose(ps_o[:, c, :], oT[:, c * 128:(c + 1) * 128],
                                        id128f_s[:])
                o_sb = ldB.tile([128, 4, 128], f32, tag="o_sb")
                nc.vector.tensor_copy(o_sb[:], ps_o[:])
                nc.sync.dma_start(out_r[fb], o_sb[:])

    nc.compile()
    return nc


def _host_prep(inputs):
    """Sort/shard/pack edges; build all per-core and shared input arrays."""
    nf32 = np.asarray(inputs["node_features"], np.float32)
    ef32 = np.asarray(inputs["edge_features"], np.float32)
    eidx = np.asarray(inputs["edge_indices"], np.int32)
    dst = eidx[:, 0].astype(np.int64)
    src = eidx[:, 1].astype(np.int64)

    order = np.argsort(dst, kind="stable")
    dst_s = dst[order]
    src_s = src[order]
    deg = np.bincount(dst, minlength=N_NODES).astype(np.int64)
    row_start = np.zeros(N_NODES + 1, np.int64)
    np.cumsum(deg, out=row_start[1:])
    scale = np.where(deg > 0, 1.0 / np.maximum(deg, 1), 0.0).astype(np.float32)

    ef_bf = ef32.astype(BF16)

    # shared arrays
    nf_pad = np.zeros((N_PAD, D), BF16)
    nf_pad[:N_NODES] = nf32.astype(BF16)
    wn_hi = np.asarray(inputs["W_node"], np.float32).astype(BF16)
    wn_lo = (np.asarray(inputs["W_node"], np.float32)
             - wn_hi.astype(np.float32)).astype(BF16)
    gamma = np.asarray(inputs["bn_gamma"], np.float32)
    var = np.asarray(inputs["bn_var"], np.float32)
    mean = np.asarray(inputs["bn_mean"], np.float32)
    beta = np.asarray(inputs["bn_beta"], np.float32)
    bns = gamma / np.sqrt(var + BN_EPS)
    bnb = beta - mean * bns
    b_node = np.asarray(inputs["b_node"], np.float32)

    shared = {
        "nf": nf_pad,
        "wg": np.asarray(inputs["W_gate"], np.float32).astype(BF16),
        "wf": np.asarray(inputs["W_filt"], np.float32).astype(BF16),
        "wn_hi": wn_hi,
        "wn_lo": wn_lo,
        "bg_col": np.asarray(inputs["b_gate"], np.float32).reshape(D, 1),
        "bnode_col": b_node.reshape(D, 1),
        "bns_col": bns.reshape(D, 1).astype(np.float32),
        "bnb_col": bnb.reshape(D, 1).astype(np.float32),
        "bf_col": np.asarray(inputs["b_filt"], np.float32).reshape(D, 1),
        "bn4_row": np.tile(b_node, 4).reshape(1, SB).astype(BF16),
        "ones_row": np.ones((1, SB), BF16),
        "id128b": np.eye(D, dtype=np.float32).astype(BF16),
        "id128f": np.eye(D, dtype=np.float32),
    }

    in_maps = []
    for c in range(N_CORES):
        r0, r1 = c * NPC, (c + 1) * NPC
        # ---- greedy window packing: whole rows, <=SB slots, <=W rows ----
        sbs_rows = []
        cur_rows = []
        cur_slots = 0
        for r in range(r0, r1):
            dg = int(deg[r])
            assert dg <= SB
            if cur_rows and (cur_slots + dg > SB or len(cur_rows) >= W):
                sbs_rows.append(cur_rows)
                cur_rows, cur_slots = [], 0
            cur_rows.append(r)
            cur_slots += dg
        if cur_rows:
            sbs_rows.append(cur_rows)
        assert len(sbs_rows) <= SB_COUNT, f"core {c}: {len(sbs_rows)} superblocks"

        ef_idx = np.full((SB_COUNT, SB), -1, np.int64)
        rel = np.zeros((SB_COUNT, SB), np.int16)
        srcs = np.full((SB_COUNT, SB), -1, np.int64)  # -1 = pad slot
        wrow = np.full((SB_COUNT, W), OWN_PAD - 1, np.int64)  # pad -> garbage row
        wscale = np.zeros((SB_COUNT, W), np.float32)
        for si, rows in enumerate(sbs_rows):
            pos = 0
            for j, r in enumerate(rows):
                wrow[si, j] = r - r0
                wscale[si, j] = scale[r]
                e0, e1 = row_start[r], row_start[r + 1]
                n = int(e1 - e0)
                if n:
                    ef_idx[si, pos:pos + n] = order[e0:e1]
                    rel[si, pos:pos + n] = j
                    srcs[si, pos:pos + n] = src_s[e0:e1]
                    pos += n

        ef_core = np.zeros((E_CAP, D), BF16)
        flat_idx = ef_idx.reshape(-1)
        valid = flat_idx >= 0
        ef_core[valid] = ef_bf[flat_idx[valid]]

        # split-table gather indices (+1 zero-slot encoding), wrapped
        # [16, 32] per sb then replicated to 128 partitions
        def wrap16(v):  # [SB_COUNT, SB] -> [128, SB_COUNT*32]
            w16 = v.reshape(SB_COUNT, 32, 16).transpose(0, 2, 1)
            return np.tile(w16, (1, 8, 1)).transpose(1, 0, 2).reshape(
                128, SB_COUNT * 32).copy()

        valid = srcs >= 0
        gidx = np.where(valid, (srcs >> 1) + 1, 0).astype(np.int16)
        # parity-split one-hot S matrices [sb, p, c, h, w]: edge selects its
        # pair-half h = src&1; invalid/pad slots get all-zero rows
        rel_pc = rel.reshape(SB_COUNT, 4, 128).transpose(0, 2, 1)      # [sb,p,c]
        par_pc = (srcs & 1).reshape(SB_COUNT, 4, 128).transpose(0, 2, 1)
        val_pc = valid.reshape(SB_COUNT, 4, 128).transpose(0, 2, 1)
        s_host = np.zeros((SB_COUNT, 128, 4, 2, W), np.float32)
        sbi, pi, ci = np.nonzero(val_pc)
        s_host[sbi, pi, ci, par_pc[sbi, pi, ci], rel_pc[sbi, pi, ci]] = 1.0
        s_host = s_host.astype(BF16)

        nf_own_hi = np.zeros((OWN_PAD, D), BF16)
        nf_own_hi[:NPC] = nf32[r0:r1].astype(BF16)
        nf_own_lo = np.zeros((OWN_PAD, D), BF16)
        nf_own_lo[:NPC] = (nf32[r0:r1]
                           - nf_own_hi[:NPC].astype(np.float32)).astype(BF16)

        # phase-3 staging gather idx: node (local) r -> its staging row sb*W+j
        stage_idx = np.zeros(OWN_PAD, np.int32)
        for si, rows in enumerate(sbs_rows):
            for j, r in enumerate(rows):
                stage_idx[r - r0] = si * W + j
        sidx = stage_idx.reshape(OWN_PAD // D, D).T.copy()  # [p, chunk]

        m = dict(shared)
        m.update({
            "ef": ef_core,
            "nf_own_hi": nf_own_hi,
            "nf_own_lo": nf_own_lo,
            "gidx_a": wrap16(gidx).astype(np.int16),
            "s_a": s_host,
            "sidx_a": sidx.astype(np.int32),
            "wscale_a": wscale.T.copy(),
        })
        in_maps.append(m)
    return in_maps


def get_program():
    if "nc" not in _CACHE:
        _CACHE["nc"] = _build_program()
    return _CACHE["nc"]


def kernel(**inputs) -> np.ndarray:
    from concourse.bass_utils import run_bass_kernel_spmd

    nc = get_program()
    in_maps = _host_prep(inputs)
    res = run_bass_kernel_spmd(nc, in_maps, core_ids=list(range(N_CORES)))
    _CACHE["last_exec_time_ns"] = res.exec_time_ns
    out = np.concatenate(
        [np.asarray(res.results[c]["out"])[:NPC] for c in range(N_CORES)], axis=0)
    return out.astype(np.float32)
